# revision 5
# baseline (speedup 1.0000x reference)
"""Trainium2 Bass kernel for nn_CubicalModel_ISM.

Reference computes Xp = X @ p and Yp = Y @ p (X, Y: [784, 32768] f32,
p: [32768] f32) and then gathers only 100 (i, j) positions from each of the
reshaped [28, 28] images.  Only the gathered rows of X and Y ever matter:
inds1/inds2 hold 100 flat row indices each, so at most 100 unique rows of X
and 100 unique rows of Y (of 784) are needed.  The host computes the unique
row sets from the (integer, data-independent) index tensors, gathers those
rows, and the device only streams ~25 MB instead of ~205 MB.

Sharding: q (parameter) axis split across 8 NeuronCores, 4096 columns each.
Per core the host packs one DRAM tensor sel[100, 8192] whose partition line
is [X_row_r | Y_row_r] for its q-shard (32 KB contiguous per partition).
The device broadcasts its p shard across the 100 partitions with a rank-1
ones-matmul (PE + ScalarE PSUM->SBUF copies), then streams sel in
[100, 1024] chunks through a quad-buffered pool, reducing each chunk with a
fused multiply + free-axis reduce (scalar_tensor_tensor accum_out) on the
DVE into one accumulator column per chunk.  Per-core partial sums
out[100, 8] are summed over cores and chunk columns on the host, and the
tiny 100-element gathers (unique-inverse mapping) finish on the host.
"""

import numpy as np

H = W = 28
Q = 32768
N_CORES = 8
QS = Q // N_CORES  # 4096 per-core q shard
NR = 100           # row capacity per tensor (= max unique gather rows)
CW = 1024          # DMA/reduce chunk width (4 KB lines)
NCH = 2 * QS // CW  # 8 chunks: 4 X + 4 Y

_CACHE = {}


def _build_nc():
    import concourse.bacc as bacc
    import concourse.mybir as mybir
    from concourse.tile import TileContext

    nc = bacc.Bacc(None)
    f32 = mybir.dt.float32
    bf16 = mybir.dt.bfloat16
    sel = nc.dram_tensor("sel", [NR, 2 * QS], f32, kind="ExternalInput")
    # p shard split into bf16 hi/lo parts on the host: ph[0] = bf16(p),
    # ph[1] = bf16(p - f32(ph[0])).  hi + lo reconstructs p to ~2^-18.
    ph = nc.dram_tensor("ph", [2, QS], bf16, kind="ExternalInput")
    out = nc.dram_tensor("out", [NR, NCH], f32, kind="ExternalOutput")

    BANK = 512  # f32 elems per PSUM bank per partition

    with TileContext(nc) as tc:
        with (
            tc.tile_pool(name="pbpool", bufs=1) as pb_pool,
            tc.tile_pool(name="chunks", bufs=6) as chunk_pool,
            tc.tile_pool(name="scratch", bufs=1) as scratch_pool,
            tc.tile_pool(name="respool", bufs=1) as res_pool,
            tc.tile_pool(name="psum", bufs=1, space="PSUM") as psum_pool,
        ):
            p_row = pb_pool.tile([2, QS], bf16)
            ones = pb_pool.tile([2, NR], bf16)
            nc.sync.dma_start(out=p_row[:, :], in_=ph[:, :])
            # Broadcast p across the 100 partitions with a K=2 rank-2 bf16
            # matmul: ones[2,100].T @ [p_hi; p_lo][2,512] -> [100,512] per
            # PSUM bank (f32 accumulation adds hi+lo back together).  bf16
            # streams at 1 cycle/row on the PE vs 4 for fp32, and the DVE
            # reads pb directly from PSUM, so there is no copy chain.
            nc.vector.memset(ones[:, :], 1.0)
            pbp = psum_pool.tile([NR, QS], f32)
            for k in range(QS // BANK):
                nc.tensor.matmul(
                    pbp[:, k * BANK : (k + 1) * BANK],
                    ones[:, :],
                    p_row[:, k * BANK : (k + 1) * BANK],
                    start=True,
                    stop=True,
                )

            res = res_pool.tile([NR, NCH], f32)
            scratch = scratch_pool.tile([NR, CW], f32)
            for k in range(NCH):
                chunk = chunk_pool.tile([NR, CW], f32, tag="chunk")
                # Alternate chunk DMA dispatch between the two HWDGE queues
                # (Sync and Scalar) so descriptor feed is not serialized
                # behind the ph DMA on one queue.
                eng = nc.scalar if k % 2 == 0 else nc.sync
                eng.dma_start(out=chunk[:, :], in_=sel[:, k * CW : (k + 1) * CW])
                pb_off = (k * CW) % QS
                # out = (chunk * 1.0) * pb elementwise (into scratch,
                # discarded); accum_out = per-partition sum.
                nc.vector.scalar_tensor_tensor(
                    out=scratch[:, :],
                    in0=chunk[:, :],
                    scalar=1.0,
                    in1=pbp[:, pb_off : pb_off + CW],
                    op0=mybir.AluOpType.mult,
                    op1=mybir.AluOpType.mult,
                    accum_out=res[:, k : k + 1],
                )
            nc.sync.dma_start(out=out[:, :], in_=res[:, :])
    nc.finalize()
    return nc


def _get_nc():
    if "nc" not in _CACHE:
        _CACHE["nc"] = _build_nc()
    return _CACHE["nc"]


def _unique_rows(inds):
    # inds: [200] int, pairs (i, j); flat row index = i*28 + j into the
    # row-major [784]-row matvec output.
    ij = np.asarray(inds).reshape(-1, 2).astype(np.int64)
    flat = ij[:, 0] * W + ij[:, 1]  # [100]
    uniq, inv = np.unique(flat, return_inverse=True)
    rows = np.full(NR, uniq[0], dtype=np.int64)
    rows[: len(uniq)] = uniq
    return rows, inv


def _make_in_maps(X, Y, p, rows1, rows2):
    import ml_dtypes

    bf16 = ml_dtypes.bfloat16
    p_hi = p.astype(bf16)
    p_lo = (p - p_hi.astype(np.float32)).astype(bf16)
    Xs = X[rows1]  # [NR, Q]
    Ys = Y[rows2]
    in_maps = []
    for c in range(N_CORES):
        sl = slice(c * QS, (c + 1) * QS)
        buf = np.empty((NR, 2 * QS), dtype=np.float32)
        buf[:, :QS] = Xs[:, sl]
        buf[:, QS:] = Ys[:, sl]
        ph = np.empty((2, QS), dtype=bf16)
        ph[0] = p_hi[sl]
        ph[1] = p_lo[sl]
        in_maps.append({"sel": buf, "ph": ph})
    return in_maps


def kernel(X, Y, p, inds1, inds2):
    from concourse.bass_utils import run_bass_kernel_spmd

    X = np.asarray(X, dtype=np.float32)
    Y = np.asarray(Y, dtype=np.float32)
    p = np.asarray(p, dtype=np.float32)

    rows1, inv1 = _unique_rows(inds1)
    rows2, inv2 = _unique_rows(inds2)

    nc = _get_nc()
    results = run_bass_kernel_spmd(
        nc, _make_in_maps(X, Y, p, rows1, rows2), list(range(N_CORES))
    ).results

    acc = np.zeros((NR, NCH), dtype=np.float32)
    for c in range(N_CORES):
        acc += results[c]["out"]
    half = NCH // 2
    xsel = acc[:, :half].sum(axis=1)  # [NR] dot(X[rows1[r]], p)
    ysel = acc[:, half:].sum(axis=1)

    dgm1 = xsel[inv1].reshape(-1, 2).astype(np.float32, copy=False)
    dgm2 = ysel[inv2].reshape(-1, 2).astype(np.float32, copy=False)
    return dgm1, dgm2


# revision 6
# speedup vs baseline: 1.1433x; 1.1433x over previous
"""Trainium2 Bass kernel for nn_CubicalModel_ISM.

Reference computes Xp = X @ p and Yp = Y @ p (X, Y: [784, 32768] f32,
p: [32768] f32) and then gathers only 100 (i, j) positions from each of the
reshaped [28, 28] images.  Only the gathered rows of X and Y ever matter:
inds1/inds2 hold 100 flat row indices each, so at most 100 unique rows of X
and 100 unique rows of Y (of 784) are needed.  The host computes the unique
row sets from the (integer, data-independent) index tensors, gathers those
rows, and the device only streams ~25 MB instead of ~205 MB.

Sharding: q (parameter) axis split across 8 NeuronCores, 4096 columns each.
Per core the host packs one DRAM tensor sel[nr, 8192] (nr = max unique rows,
typically ~94) whose partition line is [X_row_r | Y_row_r] for its q-shard
(32 KB contiguous per partition).  The p shard is shipped as bf16 hi/lo
halves and broadcast across the nr partitions with K=2 rank-2 bf16 matmuls
(1 cycle/row on the PE; f32 PSUM accumulation reconstructs p to ~2^-18).
The DVE streams sel in [nr, 1024] chunks (quad-buffered pool, single Sync
HWDGE queue) with a fused multiply + free-axis reduce
(scalar_tensor_tensor accum_out) reading pb directly from PSUM - no
PSUM->SBUF copy chain.  Per-core partial sums out[nr, 8] are summed over
cores and chunk columns on the host, and the tiny 100-element gathers
(unique-inverse mapping) finish on the host.
"""

import numpy as np

H = W = 28
Q = 32768
N_CORES = 8
QS = Q // N_CORES  # 4096 per-core q shard
CW = 1024          # DMA/reduce chunk width (4 KB lines)
NCH = 2 * QS // CW  # 8 chunks: 4 X + 4 Y

_CACHE = {}


def _build_nc(nr):
    import concourse.bacc as bacc
    import concourse.mybir as mybir
    from concourse.tile import TileContext

    nc = bacc.Bacc(None)
    f32 = mybir.dt.float32
    bf16 = mybir.dt.bfloat16
    sel = nc.dram_tensor("sel", [nr, 2 * QS], f32, kind="ExternalInput")
    # p shard split into bf16 hi/lo parts on the host: ph[0] = bf16(p),
    # ph[1] = bf16(p - f32(ph[0])).  hi + lo reconstructs p to ~2^-18.
    ph = nc.dram_tensor("ph", [2, QS], bf16, kind="ExternalInput")
    out = nc.dram_tensor("out", [nr, NCH], f32, kind="ExternalOutput")

    BANK = 512  # f32 elems per PSUM bank per partition

    with TileContext(nc) as tc:
        with (
            tc.tile_pool(name="pbpool", bufs=1) as pb_pool,
            tc.tile_pool(name="chunks", bufs=6) as chunk_pool,
            tc.tile_pool(name="scratch", bufs=1) as scratch_pool,
            tc.tile_pool(name="respool", bufs=1) as res_pool,
            tc.tile_pool(name="psum", bufs=1, space="PSUM") as psum_pool,
        ):
            p_row = pb_pool.tile([2, QS], bf16)
            ones = pb_pool.tile([2, nr], bf16)
            # ph on the Scalar HWDGE queue: runs in parallel with the sel
            # chunk dispatches on the Sync queue, so the PE broadcast starts
            # as early as possible.
            nc.scalar.dma_start(out=p_row[:, :], in_=ph[:, :])
            # Broadcast p across the nr partitions with a K=2 rank-2 bf16
            # matmul: ones[2,nr].T @ [p_hi; p_lo][2,512] -> [nr,512] per
            # PSUM bank (f32 accumulation adds hi+lo back together).  bf16
            # streams at 1 cycle/row on the PE vs 4 for fp32, and the DVE
            # reads pb directly from PSUM, so there is no copy chain.
            nc.vector.memset(ones[:, :], 1.0)
            pbp = psum_pool.tile([nr, QS], f32)
            for k in range(QS // BANK):
                nc.tensor.matmul(
                    pbp[:, k * BANK : (k + 1) * BANK],
                    ones[:, :],
                    p_row[:, k * BANK : (k + 1) * BANK],
                    start=True,
                    stop=True,
                )

            res = res_pool.tile([nr, NCH], f32)
            scratch = scratch_pool.tile([nr, CW], f32)
            for k in range(NCH):
                chunk = chunk_pool.tile([nr, CW], f32, tag="chunk")
                nc.sync.dma_start(out=chunk[:, :], in_=sel[:, k * CW : (k + 1) * CW])
                pb_off = (k * CW) % QS
                # out = (chunk * 1.0) * pb elementwise (into scratch,
                # discarded); accum_out = per-partition sum.
                nc.vector.scalar_tensor_tensor(
                    out=scratch[:, :],
                    in0=chunk[:, :],
                    scalar=1.0,
                    in1=pbp[:, pb_off : pb_off + CW],
                    op0=mybir.AluOpType.mult,
                    op1=mybir.AluOpType.mult,
                    accum_out=res[:, k : k + 1],
                )
            nc.sync.dma_start(out=out[:, :], in_=res[:, :])
    nc.finalize()
    return nc


def _get_nc(nr):
    if nr not in _CACHE:
        _CACHE[nr] = _build_nc(nr)
    return _CACHE[nr]


def _unique_rows(inds):
    # inds: [200] int, pairs (i, j); flat row index = i*28 + j into the
    # row-major [784]-row matvec output.
    ij = np.asarray(inds).reshape(-1, 2).astype(np.int64)
    flat = ij[:, 0] * W + ij[:, 1]  # [100]
    return np.unique(flat, return_inverse=True)


def _make_in_maps(X, Y, p, rows1, rows2, nr):
    import ml_dtypes

    bf16 = ml_dtypes.bfloat16
    p_hi = p.astype(bf16)
    p_lo = (p - p_hi.astype(np.float32)).astype(bf16)
    r1 = np.full(nr, rows1[0], dtype=np.int64)
    r1[: len(rows1)] = rows1
    r2 = np.full(nr, rows2[0], dtype=np.int64)
    r2[: len(rows2)] = rows2
    Xs = X[r1]  # [nr, Q]
    Ys = Y[r2]
    in_maps = []
    for c in range(N_CORES):
        sl = slice(c * QS, (c + 1) * QS)
        buf = np.empty((nr, 2 * QS), dtype=np.float32)
        buf[:, :QS] = Xs[:, sl]
        buf[:, QS:] = Ys[:, sl]
        ph = np.empty((2, QS), dtype=bf16)
        ph[0] = p_hi[sl]
        ph[1] = p_lo[sl]
        in_maps.append({"sel": buf, "ph": ph})
    return in_maps


def kernel(X, Y, p, inds1, inds2):
    from concourse.bass_utils import run_bass_kernel_spmd

    X = np.asarray(X, dtype=np.float32)
    Y = np.asarray(Y, dtype=np.float32)
    p = np.asarray(p, dtype=np.float32)

    uniq1, inv1 = _unique_rows(inds1)
    uniq2, inv2 = _unique_rows(inds2)
    nr = max(len(uniq1), len(uniq2))

    nc = _get_nc(nr)
    results = run_bass_kernel_spmd(
        nc, _make_in_maps(X, Y, p, uniq1, uniq2, nr), list(range(N_CORES))
    ).results

    acc = np.zeros((nr, NCH), dtype=np.float32)
    for c in range(N_CORES):
        acc += results[c]["out"]
    half = NCH // 2
    xsel = acc[:, :half].sum(axis=1)  # [nr]; row r = dot(X[r1[r]], p)
    ysel = acc[:, half:].sum(axis=1)

    dgm1 = xsel[inv1].reshape(-1, 2).astype(np.float32, copy=False)
    dgm2 = ysel[inv2].reshape(-1, 2).astype(np.float32, copy=False)
    return dgm1, dgm2
